# revision 13
# baseline (speedup 1.0000x reference)
"""Multi-head attention Trainium2 kernel (B=4, S=2048, E=1024, H=16, D=64).

Sharding: head-parallel x data-parallel. Core c owns heads {2c, 2c+1} for all
4 batches -> 8 (batch, head) jobs per core, no cross-core communication.

Per (batch, head) job on device (all fp32):
  qT = (Wq_aug/8)^T @ xT_aug          [64, 2048]   (bias via ones-row in xT_aug)
  kT = Wk_aug^T @ xT_aug              [64, 2048]
  v  = xT_aug^T @ Wv_aug              [2048, 64]   (+ ones column -> [.., 65])
  scoresT[k, q] = kT_chunk^T @ qT     [128, 512] tiles  (= (q . k)/8 transposed)
  attnT = exp(scoresT)                wide ACT ops, [128, 4096] per 2 k-chunks
  outT[65, q] += v_aug_chunk^T @ attnT   accumulated over 16 k-chunks in PSUM;
                                          row 64 = sum_k attnT = softmax denom
  out = outT[0:64] * (1/outT[64])     reciprocal + K=1 ones-matmul broadcast
Host side only reshapes/transposes (sharding + unsharding), no math besides
the bias/scale folding into the weight matrices.
"""

import numpy as np

import concourse.bass as bass
import concourse.mybir as mybir
import concourse.tile as tile
from concourse.bass_utils import run_bass_kernel_spmd

F32 = mybir.dt.float32
F32R = mybir.dt.float32r

B, S, E, H = 4, 2048, 1024, 16
D = E // H            # 64
NCORES = 8
HPC = H // NCORES     # heads per core = 2
PAIRS = B * HPC       # jobs per core = 8
QG = 4                # q groups of 512
NQ = S // QG          # 512
KC = S // 128         # 16 k chunks of 128
GRP = 2               # k-chunks per exp group
NGRP = KC // GRP      # 8 exp groups per job


def _patched_drain_and_barrier(self, tick_clock, wait_clock):
    # This walrus build rejects >1 sync-wait on a Drain (CTRL) instruction.
    # Collect the TileContext-exit waits on individual NOPs instead.
    nc = self.nc
    collector = nc.sync.nop(nofuse=True)
    wait_clock.add_sem_waits(
        collector.ins, tile.ScopedClock({None: tick_clock.global_clock})
    )
    si = collector.ins.sync_info
    if si is not None and len(si.on_wait) > 1:
        waits = list(si.on_wait)
        collector.ins.sync_info = mybir.SyncInfo(
            on_wait=[waits[0]], on_update=list(si.on_update)
        )
        for w in waits[1:]:
            n2 = nc.sync.nop(nofuse=True)
            n2.ins.sync_info = mybir.SyncInfo(on_wait=[w], on_update=[])
    nc.sync.drain()
    nc.all_engine_barrier()
    popped = nc._tile_sem_poison_stack.pop()
    assert popped is self._sem_poison
    nc.clear_and_free_semaphores(list(self.sems.allocated().values()))
    nc.all_engine_barrier()


tile.TileContext._drain_and_barrier = _patched_drain_and_barrier

# Consecutive matmuls share stationary operands; the default
# --enable-ldw-opt=false forces a weight reload per matmul, serializing
# LDWEIGHTS with every MM. Enable the walrus LDW dedup.
from concourse import bass_utils as _bu  # noqa: E402

if not getattr(_bu, "_ldwopt_patched", False):
    _orig_run_command = _bu.run_command

    def _run_command_ldwopt(argv, **kw):
        import os
        if os.environ.get("BASS_LDWOPT", "1") == "1":
            argv = [
                "--enable-ldw-opt=true" if a == "--enable-ldw-opt=false" else a
                for a in argv
            ]
        return _orig_run_command(argv, **kw)

    _bu.run_command = _run_command_ldwopt
    _bu._ldwopt_patched = True

_MAX_WAITS = 1


def _split_excess_waits(nc):
    """This walrus build allows at most one sync-wait per instruction; hoist
    extra waits onto NOPs inserted immediately before, on the same engine."""
    n = 0
    for f in nc.m.functions:
        for bb in f.blocks:
            new_insts = []
            for inst in bb.instructions:
                si = inst.sync_info
                if si is not None and len(si.on_wait) > _MAX_WAITS:
                    waits = list(si.on_wait)
                    for w in waits[:-_MAX_WAITS]:
                        nop = mybir.InstNoOp(
                            name=f"waitnop-{n}",
                            engine=inst.engine,
                            ins=[],
                            outs=[],
                            sync_info=mybir.SyncInfo(on_wait=[w], on_update=[]),
                            bass_nofuse=True,
                        )
                        n += 1
                        new_insts.append(nop)
                    inst.sync_info = mybir.SyncInfo(
                        on_wait=waits[-_MAX_WAITS:],
                        on_update=list(si.on_update),
                    )
                new_insts.append(inst)
            bb.instructions = new_insts


_NC_CACHE = {}


def build_nc():
    if "nc" in _NC_CACHE:
        return _NC_CACHE["nc"]
    nc = bass.Bass()
    xt = nc.dram_tensor("xt", [PAIRS, D + 1, S], F32R, kind="ExternalInput")
    wq = nc.dram_tensor("wq", [HPC, D + 1, D], F32R, kind="ExternalInput")
    wk = nc.dram_tensor("wk", [HPC, D + 1, D], F32R, kind="ExternalInput")
    wv = nc.dram_tensor("wv", [HPC, D + 1, D], F32R, kind="ExternalInput")
    out = nc.dram_tensor("out", [PAIRS, D, S], F32, kind="ExternalOutput")

    QH = S // 2  # 1024: half the q range, so PSUM fits double-buffered scores

    with tile.TileContext(nc) as tc:
        with (
            tc.tile_pool(name="sb", bufs=2) as sb,
            tc.tile_pool(name="at", bufs=3) as atp,
            tc.tile_pool(name="wp", bufs=2) as wp,
            tc.tile_pool(name="cp", bufs=1) as cp,
            tc.tile_pool(name="sp", bufs=3, space="PSUM") as sp,
            tc.tile_pool(name="op", bufs=1, space="PSUM") as op,
        ):
            # ones rows 0..64 so that ones[64:65, :] has base partition 64
            # (must match the rhs base partition in the broadcast matmul)
            ones = cp.tile([D + 1, D], F32R, tag="ones")
            nc.vector.memset(ones[:].bitcast(F32), 1.0)

            def load_pair(p):
                # inputs go on the sync queue; outputs use gpsimd so a
                # pending output DMA never blocks the next pair's prefetch
                jj = p % HPC
                xt_t = sb.tile([D + 1, S], F32R, tag="xt")
                nc.sync.dma_start(xt_t[:], xt[p])
                wq_t = wp.tile([D + 1, D], F32R, tag="wq")
                nc.sync.dma_start(wq_t[:], wq[jj])
                wk_t = wp.tile([D + 1, D], F32R, tag="wk")
                nc.sync.dma_start(wk_t[:], wk[jj])
                wv_t = wp.tile([D + 1, D], F32R, tag="wv")
                nc.sync.dma_start(wv_t[:], wv[jj])
                return xt_t, wq_t, wk_t, wv_t

            cur = load_pair(0)
            for p in range(PAIRS):
                xt_t, wq_t, wk_t, wv_t = cur

                # ---- projections (psum tiles share the "s" slots) ----
                qt = sb.tile([D, S], F32R, tag="qt")
                kt = sb.tile([D, S], F32R, tag="kt")
                for qg in range(QG):
                    sl = bass.ts(qg, NQ)
                    ps_q = sp.tile([128, 2 * NQ], F32, tag="s")
                    nc.tensor.matmul(ps_q[:D, :NQ], wq_t[:], xt_t[:, sl],
                                     start=True, stop=True)
                    nc.tensor.matmul(ps_q[:D, NQ:], wk_t[:], xt_t[:, sl],
                                     start=True, stop=True)
                    nc.vector.tensor_copy(qt[:, sl], ps_q[:D, :NQ])
                    nc.vector.tensor_copy(kt[:, sl], ps_q[:D, NQ:])

                # v with ones column: [128, 16*65]
                v_t = sb.tile([128, KC * (D + 1)], F32R, tag="v")
                nc.vector.memset(v_t[:].bitcast(F32), 1.0)
                for kc2 in range(KC // 2):
                    ps_v = sp.tile([128, 2 * NQ], F32, tag="s")
                    for h2 in range(2):
                        kc = 2 * kc2 + h2
                        nc.tensor.matmul(ps_v[:, h2 * NQ: h2 * NQ + D],
                                         xt_t[:, bass.ts(kc, 128)], wv_t[:],
                                         start=True, stop=True)
                        nc.vector.tensor_copy(
                            v_t[:, kc * (D + 1): kc * (D + 1) + D],
                            ps_v[:, h2 * NQ: h2 * NQ + D])

                # prefetch next pair's inputs while this pair computes
                if p + 1 < PAIRS:
                    cur = load_pair(p + 1)

                # ---- attention, one q-half at a time ----
                # software-pipelined: scores(kc+1) issue before out(kc) so
                # the PE never stalls on exp(kc)
                for qh in range(2):
                    q0 = qh * QH
                    out_ps = op.tile([D + 1, QH], F32, tag="out")
                    pend = None
                    for kc in range(KC):
                        ksl = bass.ts(kc, 128)
                        sps = sp.tile([128, 2 * NQ], F32, tag="s")
                        nc.tensor.matmul(sps[:, :NQ], kt[:, ksl],
                                         qt[:, q0: q0 + NQ],
                                         start=True, stop=True)
                        nc.tensor.matmul(sps[:, NQ:], kt[:, ksl],
                                         qt[:, q0 + NQ: q0 + 2 * NQ],
                                         start=True, stop=True)
                        at = atp.tile([128, 2 * NQ], F32R, tag="attn")
                        nc.scalar.activation(at[:], sps[:],
                                             mybir.ActivationFunctionType.Exp)
                        if pend is not None:
                            pat, pkc = pend
                            vsl = v_t[:, pkc * (D + 1): (pkc + 1) * (D + 1)]
                            nc.tensor.matmul(out_ps[:, :NQ], vsl, pat[:, :NQ],
                                             start=(pkc == 0), stop=False)
                            nc.tensor.matmul(out_ps[:, NQ:], vsl, pat[:, NQ:],
                                             start=(pkc == 0), stop=False)
                        pend = (at, kc)
                    pat, pkc = pend
                    vsl = v_t[:, pkc * (D + 1): (pkc + 1) * (D + 1)]
                    nc.tensor.matmul(out_ps[:, :NQ], vsl, pat[:, :NQ],
                                     start=False, stop=True)
                    nc.tensor.matmul(out_ps[:, NQ:], vsl, pat[:, NQ:],
                                     start=False, stop=True)

                    # ---- normalize: out[0:64] * (1 / out[64]) ----
                    o_t = sb.tile([D, QH], F32, tag="o")
                    for h2 in range(2):
                        sl = bass.ts(h2, NQ)
                        # denominators to SBUF (matmul rhs must be SBUF)
                        dn = sb.tile([D + 1, NQ], F32R, tag="dn")
                        nc.vector.tensor_copy(dn[D:D + 1, :],
                                              out_ps[D:D + 1, sl])
                        bc = sp.tile([128, 2 * NQ], F32, tag="s")
                        nc.tensor.matmul(bc[:D, :NQ], ones[D:D + 1, :],
                                         dn[D:D + 1, :], start=True, stop=True)
                        bc_sb = sb.tile([D, NQ], F32, tag="bc")
                        nc.vector.reciprocal(bc_sb[:], bc[:D, :NQ])
                        nc.vector.tensor_mul(o_t[:, sl], out_ps[:D, sl],
                                             bc_sb[:])
                    nc.gpsimd.dma_start(out[p, :, q0: q0 + QH], o_t[:])

    _split_excess_waits(nc)
    _NC_CACHE["nc"] = nc
    return nc


def _prep_inputs(sequences, Wq, bq, Wk, bk, Wv, bv):
    x = np.ascontiguousarray(np.asarray(sequences, dtype=np.float32))
    xh = x.reshape(B, S, H, D).transpose(2, 0, 3, 1)      # [H, B, D, S]
    aug = np.concatenate(
        [xh, np.ones((H, B, 1, S), np.float32)], axis=2)  # [H, B, 65, S]

    def augw(w, b_, scale=1.0):
        w = np.asarray(w, dtype=np.float32)
        b_ = np.asarray(b_, dtype=np.float32)
        return (np.concatenate([w, b_[:, None, :]], axis=1) * scale).astype(
            np.float32)

    wq_a = augw(Wq, bq, 1.0 / np.sqrt(D))                 # [H, 65, 64]
    wk_a = augw(Wk, bk)
    wv_a = augw(Wv, bv)

    in_maps = []
    for c in range(NCORES):
        xt_core = np.ascontiguousarray(np.stack(
            [aug[HPC * c + j, b] for b in range(B) for j in range(HPC)]))
        in_maps.append({
            "xt": xt_core,
            "wq": np.ascontiguousarray(wq_a[HPC * c: HPC * (c + 1)]),
            "wk": np.ascontiguousarray(wk_a[HPC * c: HPC * (c + 1)]),
            "wv": np.ascontiguousarray(wv_a[HPC * c: HPC * (c + 1)]),
        })
    return in_maps


def _assemble(results):
    out = np.empty((B, S, E), np.float32)
    for c in range(NCORES):
        r = results[c]["out"]                              # [8, 64, 2048]
        for b in range(B):
            for j in range(HPC):
                h = HPC * c + j
                out[b, :, h * D:(h + 1) * D] = r[HPC * b + j].T
    return out


def run(trace=False, **inputs):
    nc = build_nc()
    in_maps = _prep_inputs(**inputs)
    res = run_bass_kernel_spmd(nc, in_maps, list(range(NCORES)), trace=trace)
    return _assemble(res.results), res


def kernel(**inputs):
    out, _ = run(trace=False, **inputs)
    return out
